# revision 20
# baseline (speedup 1.0000x reference)
"""Chamfer-split loss kernel for Trainium2 (8 NeuronCores, data-parallel over batch).

Per item: d2[n,m] = ||t_n||^2 + ||r_m||^2 - 2 t_n.r_m.  The PE computes
neg_q[n,m] = 2*cross - rm2' via K=5 float32r matmuls (4 coordinate rows plus a
penalty row rm2' = rm2 + BIG*(pid==0)); then min_m d2'[n] = tn2[n] - max_m
neg_q[n,:] (sqrt is monotone so the min is taken on squared distances).  The
two chamfer directions are the two matmul orientations.  Per-item sums come
from ones-matmuls; the final ~10 flops/item run on host from a [128,3] output.

Hardware constraints shaping the layout:
- matmul operands must start at partition 0/32/64 with equal bases, so
  transposed operand groups sit at a 32-row pitch, 3 items per PE transpose,
  blocked by (item-block, chunk); column order is j = c*32 + b.
- walrus embeds at most ONE semaphore wait per instruction, so ops that read
  DMA-written tiles are split per-chunk (one DMA dep each), all prep runs on
  the vector engine, and a dummy eye-transpose absorbs the eye DMA wait on PE.
"""

import os
import sys

sys.path.insert(0, "/opt/trn_rl_repo")

KSTAGE = int(os.environ.get("KSTAGE", "3"))

import numpy as np

import concourse.bass as bass
import concourse.mybir as mybir
from concourse.tile import TileContext, add_dep_helper

B, N, M, D = 256, 256, 256, 4
NCORES = 8
PER = B // NCORES  # 32 items per core
C = 2              # 128-row chunks per item
BC = PER * C       # 64 (chunk, item) columns per core
P = 128
BIG = 1e10
F32 = mybir.dt.float32
F32R = mybir.dt.float32r
I32 = mybir.dt.int32
AX = mybir.AxisListType
ALU = mybir.AluOpType

PITCH = 32          # operand group pitch (matmul base-partition alignment)
GPT = 3             # groups (items) per transpose (bases 0/32/64)
RG = 8              # matmul tiles per PSUM reduce group


def _prep(nc, natB, natA, pid_sb, eq, mask, x2, sq, tmpa, tmpb):
    """masks, squared norms, penalty col for one tensor side (all on DVE)."""
    v = nc.vector
    natB_f = natB[:].rearrange("p c b x -> p (c b) x")
    natA_f = natA[:].rearrange("p c b x -> p (c b) x")
    for c in range(C):
        cs = slice(c * PER, (c + 1) * PER)
        v.tensor_scalar(eq[:, cs], pid_sb[:, cs], 0, None, ALU.is_equal)
        v.tensor_scalar(mask[:, cs], pid_sb[:, cs], 0, None, ALU.not_equal)
    for c in range(C):
        v.tensor_tensor(sq[:, c * PER:(c + 1) * PER, :],
                        natB[:, c, :, 0:4], natB[:, c, :, 0:4], op=ALU.mult)
    v.tensor_reduce(x2[:], sq[:], axis=AX.X, op=ALU.add)
    v.tensor_scalar(tmpa[:], eq[:], -BIG, None, ALU.mult)
    v.tensor_scalar(tmpb[:], x2[:], -1.0, None, ALU.mult)
    v.tensor_tensor(natB_f[:, :, 4], tmpa[:], tmpb[:], op=ALU.add)
    for c in range(C):
        v.tensor_copy(natA[:, c, :, 0:4], natB[:, c, :, 0:4])


def build_nc():
    nc = bass.Bass()

    tgt = nc.dram_tensor("tgt", [PER, N, D], F32, kind="ExternalInput")
    rec = nc.dram_tensor("rec", [PER, M, D], F32, kind="ExternalInput")
    ipid = nc.dram_tensor("ipid", [PER, N], I32, kind="ExternalInput")
    opid = nc.dram_tensor("opid", [PER, M], I32, kind="ExternalInput")
    eye = nc.dram_tensor("eye", [P, P], F32, kind="ExternalInput")
    out = nc.dram_tensor("out", [P, 3], F32, kind="ExternalOutput")

    n_bblk = (PER + GPT - 1) // GPT   # 11 item-blocks

    with TileContext(nc) as tc:
        with (
            tc.tile_pool(name="nat", bufs=1) as nat_pool,
            tc.tile_pool(name="sm", bufs=1) as sm_pool,
            tc.tile_pool(name="small", bufs=1) as small,
        ):
            natB_t = nat_pool.tile([P, C, PER, PITCH], F32, tag="nbt")
            natB_r = nat_pool.tile([P, C, PER, PITCH], F32, tag="nbr")
            natA_t = nat_pool.tile([P, C, PER, PITCH], F32, tag="nat")
            natA_r = nat_pool.tile([P, C, PER, PITCH], F32, tag="nar")
            ipid_sb = small.tile([P, BC], I32, tag="ipid")
            opid_sb = small.tile([P, BC], I32, tag="opid")
            eye_sb = small.tile([P, P], F32, tag="eye")

            nc.sync.dma_start(eye_sb[:], eye[:])
            for cc in range(C):
                nc.sync.dma_start(natB_t[:, cc, :, 0:4],
                                  tgt[:].rearrange("b (c p) d -> p c b d", p=P)[:, cc])
                nc.sync.dma_start(natB_r[:, cc, :, 0:4],
                                  rec[:].rearrange("b (c p) d -> p c b d", p=P)[:, cc])
            for cc in range(C):
                nc.sync.dma_start(ipid_sb[:, cc * PER:(cc + 1) * PER],
                                  ipid[:].rearrange("b (c p) -> p c b", p=P)[:, cc])
                nc.sync.dma_start(opid_sb[:, cc * PER:(cc + 1) * PER],
                                  opid[:].rearrange("b (c p) -> p c b", p=P)[:, cc])

            eq_x = small.tile([P, BC], F32, tag="eqx")
            eq_y = small.tile([P, BC], F32, tag="eqy")
            mask_x = small.tile([P, BC], F32, tag="mx")
            mask_y = small.tile([P, BC], F32, tag="my")
            t2 = small.tile([P, BC], F32, tag="t2")
            r2 = small.tile([P, BC], F32, tag="r2")
            sq = small.tile([P, BC, D], F32, tag="sq2")
            sq_r = small.tile([P, BC, D], F32, tag="sq2r")
            ta = small.tile([P, BC], F32, tag="ta")
            tb = small.tile([P, BC], F32, tag="tb")
            ta_r = small.tile([P, BC], F32, tag="tar")
            tb_r = small.tile([P, BC], F32, tag="tbr")

            # pad columns must be initialized: the transposes enumerate all 32
            # columns per group and uninitialized PSUM reads fault on hardware.
            # col 4 of the A form is the 0.5 ones-row (scaled x2 by the copy).
            for natA in (natA_t, natA_r):
                nc.gpsimd.memset(natA[:].rearrange("p c b x -> p (c b) x")[:, :, 4:PITCH], 0.5)
            for natB in (natB_t, natB_r):
                nc.gpsimd.memset(natB[:].rearrange("p c b x -> p (c b) x")[:, :, 5:PITCH], 0.0)

            _prep(nc, natB_t, natA_t, ipid_sb, eq_x, mask_x, t2, sq, ta, tb)
            _prep(nc, natB_r, natA_r, opid_sb, eq_y, mask_y, r2, sq_r, ta_r, tb_r)

            # ---- transposed operand forms (A: [2xT;1] stationary, B: [xT;-x2'] moving)
            a_sb, b_sb = {}, {}
            with (
                tc.tile_pool(name="pstr_a", bufs=3, space="PSUM") as pstr_a,
                tc.tile_pool(name="pstr_b", bufs=2, space="PSUM") as pstr_b,
                tc.tile_pool(name="pstr_d", bufs=1, space="PSUM") as pstr_d,
            ):
                # dummy transpose: absorbs the eye DMA wait on the PE engine so
                # every real transpose carries only the DVE-prep wait
                ps_dummy = pstr_d.tile([PITCH, PITCH], F32, tag="ps_dummy")
                dummy = nc.tensor.transpose(ps_dummy[:], eye_sb[0:PITCH, 0:PITCH],
                                            eye_sb[0:PITCH, 0:PITCH])
                for name, natA, natB in (("t", natA_t, natB_t), ("r", natA_r, natB_r)):
                    for k in range(n_bblk):
                        g0, g1 = k * GPT, min((k + 1) * GPT, PER)
                        rows = (g1 - g0) * PITCH
                        for c in range(C):
                            ps = pstr_a.tile([P, P], F32, tag="ps_a")
                            ti = nc.tensor.transpose(
                                ps[0:rows, :], natA[:, c, g0:g1, :], eye_sb[:])
                            add_dep_helper(ti.ins, dummy.ins, sync=False)
                            sb = sm_pool.tile([P, P], F32R, tag=f"a_{name}{k}_{c}")
                            nc.scalar.mul(sb[0:rows, :], ps[0:rows, :], 2.0)
                            a_sb[(name, k, c)] = sb
                        ps = pstr_b.tile([P, C * P], F32, tag="ps_b")
                        for c in range(C):
                            ti = nc.tensor.transpose(
                                ps[0:rows, c * P:(c + 1) * P],
                                natB[:, c, g0:g1, :], eye_sb[:])
                            add_dep_helper(ti.ins, dummy.ins, sync=False)
                        sb = sm_pool.tile([P, C * P], F32R, tag=f"b_{name}{k}")
                        last_copy = nc.scalar.copy(sb[0:rows, :], ps[0:rows, :])
                        b_sb[(name, k)] = sb

            def a_rows(name, b, c):
                t = a_sb[(name, b // GPT, c)]
                r0 = PITCH * (b % GPT)
                return t[r0:r0 + 5, :]

            def b_rows(name, b):
                t = b_sb[(name, b // GPT)]
                r0 = PITCH * (b % GPT)
                return t[r0:r0 + 5, :]

            if KSTAGE == 1:
                out_sb = small.tile([P, 3], F32, tag="outsb")
                nc.scalar.copy(out_sb[:], b_sb[("t", 0)][:, 0:3])
                nc.sync.dma_start(out[:], out_sb[:])
                return nc

            # ---- main loop: 128 matmuls in groups of RG, batched max-reduce
            mx1 = small.tile([P, BC], F32, tag="mxd1")
            mx2 = small.tile([P, BC], F32, tag="mxd2")
            # Matmuls are ordered by operand base partition: rapidly switching
            # the PE row-tile position between matmuls hangs the hardware, so
            # each base (phase) runs as one contiguous block.
            first = True
            with tc.tile_pool(name="psmm", bufs=2, space="PSUM") as psmm:
                for d in range(2):
                    sname, mname = ("t", "r") if d == 0 else ("r", "t")
                    dst = mx1 if d == 0 else mx2
                    for phase in range(GPT):
                        items = list(range(phase, PER, GPT))
                        for c in range(C):
                            for i0 in range(0, len(items), RG):
                                chunk = items[i0:i0 + RG]
                                ps = psmm.tile([P, RG, C * P], F32, tag="ps_mm")
                                for t, b in enumerate(chunk):
                                    mm = nc.tensor.matmul(
                                        ps[:, t, :],
                                        a_rows(sname, b, c),
                                        b_rows(mname, b),
                                    )
                                    if first:
                                        # PSUM banks are reused across closed pools;
                                        # Tile does not emit cross-pool hazard deps,
                                        # so gate the first matmul on the last copy.
                                        add_dep_helper(mm.ins, last_copy.ins, sync=True)
                                        first = False
                                k = len(chunk)
                                j0 = c * PER + chunk[0]
                                nc.vector.tensor_reduce(
                                    dst[:, j0:j0 + GPT * (k - 1) + 1:GPT],
                                    ps[:, 0:k, :], axis=AX.X, op=ALU.max)

            if KSTAGE == 2:
                out_sb = small.tile([P, 3], F32, tag="outsb")
                nc.scalar.copy(out_sb[:], mx1[:, 0:3])
                nc.sync.dma_start(out[:], out_sb[:])
                return nc

            # ---- epilogue: masked sqrt of min squared distances, per-item sums
            src1 = small.tile([P, P], F32, tag="src1")
            src2 = small.tile([P, P], F32, tag="src2")
            src3 = small.tile([P, P], F32, tag="src3")
            tm1 = small.tile([P, BC], F32, tag="tm1")
            tm2 = small.tile([P, BC], F32, tag="tm2")
            v1 = small.tile([P, BC], F32, tag="v1")
            v2 = small.tile([P, BC], F32, tag="v2")
            zx = small.tile([P, BC], F32, tag="zx")
            zy = small.tile([P, BC], F32, tag="zy")

            SQ = mybir.ActivationFunctionType.Sqrt
            nc.vector.tensor_tensor(tm1[:], t2[:], mx1[:], op=ALU.subtract)
            nc.vector.tensor_scalar(tm1[:], tm1[:], 0.0, None, ALU.max)
            nc.scalar.activation(v1[:], tm1[:], SQ)
            nc.vector.tensor_tensor(src1[:, 0:BC], v1[:], mask_x[:], op=ALU.mult)
            nc.vector.tensor_tensor(tm2[:], r2[:], mx2[:], op=ALU.subtract)
            nc.vector.tensor_scalar(tm2[:], tm2[:], 0.0, None, ALU.max)
            nc.scalar.activation(v2[:], tm2[:], SQ)
            nc.vector.tensor_tensor(src1[:, BC:P], v2[:], mask_y[:], op=ALU.mult)

            nc.scalar.activation(zy[:], r2[:], SQ)
            nc.vector.tensor_tensor(src2[:, 0:BC], zy[:], eq_y[:], op=ALU.mult)
            nc.vector.tensor_copy(src2[:, BC:P], eq_y[:])
            nc.scalar.activation(zx[:], t2[:], SQ)
            nc.vector.tensor_tensor(src3[:, 0:BC], zx[:], mask_x[:], op=ALU.mult)
            nc.vector.tensor_copy(src3[:, BC:P], eq_x[:])

            ones_sb = small.tile([P, 1], F32, tag="ones")
            nc.vector.memset(ones_sb[:], 1.0)
            with tc.tile_pool(name="pssum", bufs=1, space="PSUM") as pssum:
                ps_s = pssum.tile([P, 4], F32, tag="ps_sum")
                nc.tensor.matmul(ps_s[:, 0:1], src1[:], ones_sb[:])
                nc.tensor.matmul(ps_s[:, 1:2], src2[:], ones_sb[:])
                nc.tensor.matmul(ps_s[:, 2:3], src3[:], ones_sb[:])
                out_sb = small.tile([P, 3], F32, tag="outsb")
                nc.scalar.copy(out_sb[:], ps_s[:, 0:3])
            nc.sync.dma_start(out[:], out_sb[:])

    return nc


def _split_multiwaits(jb: bytes) -> bytes:
    """walrus accepts only one embedded semaphore wait per instruction; hoist
    surplus waits onto standalone EventSemaphore instructions just before."""
    import orjson
    j = orjson.loads(jb)
    ctr = 0
    for func in j["functions"]:
        for blk in func["blocks"]:
            out = []
            for inst in blk["instructions"]:
                si = inst.get("sync_info")
                waits = (si or {}).get("on_wait") or []
                if len(waits) > 1:
                    for w in waits[:-1]:
                        ctr += 1
                        out.append({"debug": 0, "engine": inst["engine"], "ins": [],
                                    "outs": [], "name": f"xwait_{ctr}",
                                    "opcode": "EventSemaphore",
                                    "sync_info": {"on_update": [], "on_wait": [w]}})
                    si["on_wait"] = [waits[-1]]
                out.append(inst)
            blk["instructions"] = out
    return orjson.dumps(j)


_CACHE = {}


def _get_nc():
    if "nc" not in _CACHE:
        nc = build_nc()
        patched = _split_multiwaits(nc.to_json_bytes())
        nc.to_json_bytes = lambda: patched
        _CACHE["nc"] = nc
    return _CACHE["nc"]


def kernel(target, reco, in_pid, out_pid):
    from concourse.bass_utils import run_bass_kernel_spmd

    nc = _get_nc()
    eye = np.eye(P, dtype=np.float32)
    in_maps = []
    for r in range(NCORES):
        sl = slice(r * PER, (r + 1) * PER)
        in_maps.append({
            "tgt": np.ascontiguousarray(target[sl]),
            "rec": np.ascontiguousarray(reco[sl]),
            "ipid": np.ascontiguousarray(in_pid[sl]),
            "opid": np.ascontiguousarray(out_pid[sl]),
            "eye": eye,
        })
    res = run_bass_kernel_spmd(nc, in_maps, list(range(NCORES)))

    # host epilogue: ~10 flops per item from the per-(chunk,item) partial sums
    # column order: j = c*PER + b
    s = np.zeros((6, B), dtype=np.float64)
    for r in range(NCORES):
        o = res.results[r]["out"].astype(np.float64)  # [128, 3]
        for b in range(PER):
            j0, j1 = b, PER + b
            gb = r * PER + b
            s[0, gb] = o[j0, 0] + o[j1, 0]            # sum_xy
            s[1, gb] = o[BC + j0, 0] + o[BC + j1, 0]  # sum_yx
            s[2, gb] = o[j0, 1] + o[j1, 1]            # sum_norm_y_zero
            s[3, gb] = o[BC + j0, 1] + o[BC + j1, 1]  # count(out_pid==0)
            s[4, gb] = o[j0, 2] + o[j1, 2]            # sum_norm_x_nz
            s[5, gb] = o[BC + j0, 2] + o[BC + j1, 2]  # count(in_pid==0)
    s1, s2, s6, cnt0y, s5, cnt0x = s
    nx = N - cnt0x
    ny = M - cnt0y
    n_in = np.maximum(1.0, nx)
    n_out = np.maximum(1.0, ny)
    normal = 0.5 * (s1 / n_out + s2 / n_in)
    eucl_nz = np.where(ny == 0, s5 / n_in, np.where(nx == 0, 0.0, normal))
    eucl_z = s6 / np.maximum(1.0, cnt0y)
    return (np.float32(eucl_nz.mean()), np.float32(eucl_z.mean()))
